# revision 36
# baseline (speedup 1.0000x reference)
"""Trainium2 Bass kernel for an autoregressive LSTMCell decoder with softmax feedback.

Math (per timestep, PyTorch gate order i,f,g,o):
    gates = [x_t, y] @ W_ih.T + b_ih + h @ W_hh.T + b_hh
    i,f,o = sigmoid(...), g = tanh(...)
    c = f*c + i*g ; h = o*tanh(c) ; y = softmax(h @ W_lin.T + b_lin)

Strategy (8 NeuronCores, data parallel over batch, 32 sequences/core):
  * Features-on-partitions: gates.T [2048, 32] packed into one PSUM bank
    [128, (q,b)=512]; cell/hidden state packed [128, (k,b)=128].
  * Gate-chunk order q = (k//2)*8 + gate*2 + (k%2) so h-feature chunks
    k in {0,1} land in PSUM cols 0:256 ("A") and k in {2,3} in 256:512
    ("B").  The ACT/DVE tail (tanh -> cell update -> H) runs per half
    and pipelines against the PE: tanh_A runs while the PE still
    accumulates B columns, and the next step's W_hh k=0,1 matmuls start
    as soon as H_A exists.
  * Recurrent weights (W_hh, W_y) are fp8-e4m3 scaled by 64 (fast
    weight load = 4 elem/cycle; x64 keeps values out of the subnormal
    range).  The x64 is undone for free by the gate tanh's scale=1/64.
  * x @ W_x.T precomputed per 32-step chunk as full-utilization f32r
    matmuls; bias is folded in during the PSUM->SBUF staging copy
    (per-partition scalar add), result kept in SBUF as fp16 (x64).
  * All gate activations via Tanh (sigmoid(x)=0.5+0.5*tanh(x/2), scales
    folded into weights host-side); softmax via Exp + reciprocal -- both
    functions live in the single "exp_and_others" ACT table set.
  * Cell state carried as S=2c, hidden as H=2h (absorbed into
    0.5-scaled W_hh / W_lin).  State math in fp16 (2x DVE mode).
  * y written to DRAM in fp16; host converts to f32.
"""

import os
import sys

sys.path.insert(0, "/opt/trn_rl_repo")

# Optional variants (measured speed-neutral on HW within noise; row-tiled
# W_y crashes the device -- keep off).
V3_GX_ACT = os.environ.get("V3_GX_ACT", "0") == "1"
V3_WY_PAIRS = os.environ.get("V3_WY_PAIRS", "0") == "1"
V3_BIAS_DVE = os.environ.get("V3_BIAS_DVE", "0") == "1"
# Diagnostic: break the recurrence (matmuls read constant state) to
# measure pure PE-stream throughput without tail stalls.  WRONG RESULTS.
PE_ONLY = os.environ.get("PE_ONLY", "0") == "1"
# Issue the next step's PE identity-inject before logits k=2,3 so the PE
# has more covering work while the B-half tail (H_B) is still in flight.
INIT_EARLY = os.environ.get("INIT_EARLY", "1") == "1"

import numpy as np
import ml_dtypes

import concourse.bass as bass  # noqa: F401
import concourse.tile as tile
from concourse import bacc, mybir
from concourse.bass_utils import run_bass_kernel_spmd
from concourse.masks import make_identity

f32 = mybir.dt.float32
f32r = mybir.dt.float32r
f16 = mybir.dt.float16
f8 = mybir.dt.float8e4
AF = mybir.ActivationFunctionType
ALU = mybir.AluOpType

B, D, N = 256, 512, 64
NCORES = 8
BL = B // NCORES  # 32 sequences per core
TC = 32           # timesteps per chunk
P = 128
SCALE = 64.0      # fp8 weight scale, undone by tanh ACT scale

# chunk order: old chunk index m = gate*4 + k  ->  position q
# q = (k//2)*8 + gate*2 + (k%2):  A half (cols 0:256) holds k in {0,1}
_PERM = [0] * 16
for _q in range(16):
    _k = (_q // 8) * 2 + (_q % 2)
    _gate = (_q % 8) // 2
    _PERM[_q] = _gate * 4 + _k

_CACHE = {}


def _build(T, reps=1):
    NCH = T // TC
    nc = bacc.Bacc("TRN2", target_bir_lowering=False, debug=False, num_devices=NCORES)

    x_d = nc.dram_tensor("x", [NCH, P, 4 * BL * TC], f32, kind="ExternalInput").ap()
    h0_d = nc.dram_tensor("h0", [BL, D], f32, kind="ExternalInput").ap()
    wxt_d = nc.dram_tensor("wxt", [P, 4 * 16 * P], f32, kind="ExternalInput").ap()
    wht_d = nc.dram_tensor("wht", [P, 4 * 16 * P], f8, kind="ExternalInput").ap()
    wyt_d = nc.dram_tensor("wyt", [N, 16 * P], f8, kind="ExternalInput").ap()
    wyp_d = nc.dram_tensor("wyp", [P, 8 * P], f8, kind="ExternalInput").ap()
    wlt_d = nc.dram_tensor("wlt", [P, 4 * N], f16, kind="ExternalInput").ap()
    bgt_d = nc.dram_tensor("bgt", [P, 16], f32, kind="ExternalInput").ap()
    bl_d = nc.dram_tensor("bl", [1, N], f16, kind="ExternalInput").ap()
    blb_d = nc.dram_tensor("blb", [BL, N], f16, kind="ExternalInput").ap()
    out_d = nc.dram_tensor("out", [BL, T, N], f16, kind="ExternalOutput").ap()

    with tile.TileContext(nc) as tc:
        with (
            tc.tile_pool(name="const", bufs=1) as const,
            tc.tile_pool(name="xst", bufs=2) as xst,
            tc.tile_pool(name="gxp", bufs=2) as gxp,
            tc.tile_pool(name="yout", bufs=2) as yout,
            tc.tile_pool(name="state", bufs=2) as state,
            tc.tile_pool(name="tmp", bufs=3) as tmp,
            tc.tile_pool(name="g_ps", bufs=2, space="PSUM") as g_psp,
            tc.tile_pool(name="pre_ps", bufs=2, space="PSUM") as pre_psp,
            tc.tile_pool(name="l_ps", bufs=2, space="PSUM") as l_psp,
        ):
            # ---- constants / weights ----
            wxt = const.tile([P, 4 * 16 * P], f32r)
            nc.sync.dma_start(out=wxt, in_=wxt_d.bitcast(f32r))
            wht = const.tile([P, 4 * 16 * P], f8)
            nc.sync.dma_start(out=wht, in_=wht_d)
            wyt = const.tile([N, 16 * P], f8)
            nc.sync.dma_start(out=wyt, in_=wyt_d)
            wyp = const.tile([P, 8 * P], f8)
            nc.sync.dma_start(out=wyp, in_=wyp_d)
            wlt = const.tile([P, 4 * N], f16)
            nc.sync.dma_start(out=wlt, in_=wlt_d)
            bgt = const.tile([P, 16], f32)
            nc.sync.dma_start(out=bgt, in_=bgt_d)
            bl = const.tile([1, N], f16)
            nc.sync.dma_start(out=bl, in_=bl_d)
            blb = const.tile([BL, N], f16)
            nc.sync.dma_start(out=blb, in_=blb_d)

            ones32 = const.tile([1, BL], f16)
            nc.vector.memset(ones32, 1.0)
            idf = const.tile([P, P], f32)
            make_identity(nc, idf)
            id8 = const.tile([P, P], f8)
            nc.vector.tensor_copy(id8, idf)

            for _rep in range(reps):
                # ---- initial state ----
                h0s = const.tile([P, 4, BL], f32)
                h0r = h0_d.rearrange("b (k p) -> k p b", p=P)
                for k in range(4):
                    nc.sync.dma_start(out=h0s[:, k, :], in_=h0r[k])
                H = state.tile([P, 4 * BL], f16, tag="H")
                nc.vector.tensor_scalar(out=H, in0=h0s.rearrange("p k b -> p (k b)"),
                                        scalar1=2.0, scalar2=None, op0=ALU.mult)
                S = state.tile([P, 4 * BL], f16, tag="S")
                nc.vector.memset(S, 0.0)
                yT2 = state.tile([P, BL], f16, tag="yT")
                nc.vector.memset(yT2, 0.0)
                if PE_ONLY:
                    H0_const = const.tile([P, 4 * BL], f16)
                    nc.vector.memset(H0_const, 0.01)
                    yT2_const = const.tile([P, BL], f16)
                    nc.vector.memset(yT2_const, 0.01)

                def stage_x(ch):
                    xT = xst.tile([P, 4 * BL * TC], f32r, tag="xT")
                    nc.sync.dma_start(out=xT, in_=x_d[ch].bitcast(f32r))
                    return xT.rearrange("p (k bt) -> p k bt", k=4)

                def pre_mms(xTv, q, half):
                    # one PSUM tile of (64*sg*W_x) @ x for gate-chunk q,
                    # batch-half `half` (all 32 timesteps of the chunk)
                    pp = pre_psp.tile([P, 512], f32, tag="pre")
                    for kx in range(4):
                        nc.tensor.matmul(
                            pp,
                            wxt[:, (kx * 16 + q) * P:(kx * 16 + q + 1) * P],
                            xTv[:, kx, half * 512:(half + 1) * 512],
                            start=(kx == 0), stop=(kx == 3), skip_group_check=True,
                        )
                    return pp

                def pre_copy(pp, GXv, q, half):
                    # permuted copy into GX as fp16, bias folded in
                    ppv = pp.rearrange("c (b t) -> c t b", t=TC)
                    nc.vector.tensor_scalar(
                        out=GXv[:, :, 32 * q + 16 * half: 32 * q + 16 * half + 16],
                        in0=ppv, scalar1=bgt[:, q:q + 1], scalar2=None, op0=ALU.add)

                def pre_group(xTv, GXv, q, half):
                    pre_copy(pre_mms(xTv, q, half), GXv, q, half)

                xTv_next = stage_x(0)
                GX = gxp.tile([P, TC * 512], f16, tag="GX")
                GXv = GX.rearrange("p (t mm) -> p t mm", mm=512)
                for q in range(16):
                    for half in range(2):
                        pre_group(xTv_next, GXv, q, half)

                for ch in range(NCH):
                    GX_cur = GX
                    if ch + 1 < NCH:
                        xTv_next = stage_x(ch + 1)
                        GX = gxp.tile([P, TC * 512], f16, tag="GX")
                        GXv = GX.rearrange("p (t mm) -> p t mm", mm=512)
                    Y = yout.tile([BL, TC * N], f16, tag="Y")
                    if PE_ONLY:
                        nc.vector.memset(Y, 0.0)
                    g_ps_next = None

                    for tt in range(TC):
                        if PE_ONLY:
                            # constant stand-ins: PE stream only, no tails
                            H = H0_const
                            yT2 = yT2_const
                        # ---------- PE: accumulate gates for step tt ----------
                        # g_ps for this step was gx-initialized either by the
                        # PE identity-inject below (tt==0, or the very first
                        # bank occupancies) or by an ACT copy issued one step
                        # early; in the latter case the matmuls accumulate on
                        # top via the has_written bits still set from this
                        # bank's previous occupancy.
                        if g_ps_next is None:
                            g_ps = g_psp.tile([P, 512], f32, tag="g")
                            nc.tensor.matmul(g_ps, id8,
                                             GX_cur[:, tt * 512:(tt + 1) * 512],
                                             start=True, stop=False,
                                             skip_group_check=True)
                        else:
                            g_ps = g_ps_next
                            g_ps_next = None
                        # kpair-major: all k=0,1 matmuls (need only H_A of the
                        # previous step) for every output chunk, then k=2,3.
                        # Within each group A-columns (q<8) come first.  This
                        # pushes the last write of each column half ~1us
                        # earlier in the PE stream so the tanh/cell tails have
                        # more covering PE work.
                        for kpair in range(2):
                            for q in range(16):
                                for k in (2 * kpair, 2 * kpair + 1):
                                    nc.tensor.matmul(
                                        g_ps[:, 32 * q:32 * q + 32],
                                        wht[:, (k * 16 + q) * P:(k * 16 + q + 1) * P],
                                        H[:, 32 * k:32 * k + 32],
                                        start=False, stop=False,
                                        skip_group_check=True,
                                    )
                        # y feedback, A half then B half (latest dep: yT2)
                        for q in range(8):
                            nc.tensor.matmul(
                                g_ps[:, 32 * q:32 * q + 32],
                                wyt[:, q * P:(q + 1) * P],
                                yT2[0:64, :],
                                start=False, stop=(q == 7),
                                skip_group_check=True,
                            )
                        if not PE_ONLY:
                            # ---- ACT tail for the A half starts now ----
                            TgA = tmp.tile([P, 256], f16, tag="TgA")
                            nc.scalar.activation(out=TgA, in_=g_ps[:, 0:256],
                                                 func=AF.Tanh, scale=1.0 / SCALE)
                        for q in range(8, 16):
                            nc.tensor.matmul(
                                g_ps[:, 32 * q:32 * q + 32],
                                wyt[:, q * P:(q + 1) * P],
                                yT2[0:64, :],
                                start=False, stop=(q == 15),
                                skip_group_check=True,
                            )
                        if not PE_ONLY:
                            # TgB queued on ACT right behind TgA (before the
                            # Tc ops) so the B tail starts as soon as the B
                            # columns land.
                            TgB = tmp.tile([P, 256], f16, tag="TgB")
                            nc.scalar.activation(out=TgB, in_=g_ps[:, 256:512],
                                                 func=AF.Tanh, scale=1.0 / SCALE)
                            # PE cover while the tails run: next chunk's
                            # precompute + next step's PSUM gx-init.
                            pp_fill = None
                            if ch + 1 < NCH:
                                pp_fill = pre_mms(xTv_next, tt // 2, tt % 2)
                            if tt + 1 < TC and not (ch == 0 and tt == 0):
                                if V3_GX_ACT:
                                    g_ps_next = g_psp.tile([P, 512], f32, tag="g")
                                    nc.scalar.copy(
                                        out=g_ps_next,
                                        in_=GX_cur[:, (tt + 1) * 512:(tt + 2) * 512])
                                elif INIT_EARLY:
                                    g_ps_next = g_psp.tile([P, 512], f32, tag="g")
                                    nc.tensor.matmul(
                                        g_ps_next, id8,
                                        GX_cur[:, (tt + 1) * 512:(tt + 2) * 512],
                                        start=True, stop=False,
                                        skip_group_check=True)
                        if PE_ONLY:
                            l_ps = l_psp.tile([BL, N], f32, tag="l")
                            nc.tensor.matmul(l_ps, ones32, bl, start=True,
                                             stop=False, skip_group_check=True)
                            for k in range(4):
                                nc.tensor.matmul(l_ps, H[:, 32 * k:32 * k + 32],
                                                 wlt[:, N * k:N * (k + 1)],
                                                 start=False, stop=(k == 3),
                                                 skip_group_check=True)
                            if ch + 1 < NCH:
                                pre_copy(pre_mms(xTv_next, tt // 2, tt % 2),
                                         GXv, tt // 2, tt % 2)
                            continue
                        # ---- DVE/ACT cell update: uvS for A then B first
                        # (TcA/TcB interleave on ACT behind TgB), H writes
                        # last so the DVE FIFO never blocks the B chain.
                        S_new = state.tile([P, 4 * BL], f16, tag="S")
                        H_new = state.tile([P, 4 * BL], f16, tag="H")
                        uA = tmp.tile([P, 2 * BL], f16, tag="uA")
                        nc.vector.scalar_tensor_tensor(
                            out=uA, in0=TgA[:, 64:128], scalar=1.0,
                            in1=S[:, 0:64], op0=ALU.add, op1=ALU.mult)
                        vA = tmp.tile([P, 2 * BL], f16, tag="vA")
                        nc.vector.scalar_tensor_tensor(
                            out=vA, in0=TgA[:, 0:64], scalar=1.0,
                            in1=TgA[:, 128:192], op0=ALU.add, op1=ALU.mult)
                        nc.vector.scalar_tensor_tensor(
                            out=S_new[:, 0:64], in0=uA, scalar=0.5,
                            in1=vA, op0=ALU.mult, op1=ALU.add)
                        TcA = tmp.tile([P, 2 * BL], f16, tag="TcA")
                        nc.scalar.activation(out=TcA, in_=S_new[:, 0:64],
                                             func=AF.Tanh, scale=0.5)
                        uB = tmp.tile([P, 2 * BL], f16, tag="uB")
                        nc.vector.scalar_tensor_tensor(
                            out=uB, in0=TgB[:, 64:128], scalar=1.0,
                            in1=S[:, 64:128], op0=ALU.add, op1=ALU.mult)
                        vB = tmp.tile([P, 2 * BL], f16, tag="vB")
                        nc.vector.scalar_tensor_tensor(
                            out=vB, in0=TgB[:, 0:64], scalar=1.0,
                            in1=TgB[:, 128:192], op0=ALU.add, op1=ALU.mult)
                        nc.vector.scalar_tensor_tensor(
                            out=S_new[:, 64:128], in0=uB, scalar=0.5,
                            in1=vB, op0=ALU.mult, op1=ALU.add)
                        TcB = tmp.tile([P, 2 * BL], f16, tag="TcB")
                        nc.scalar.activation(out=TcB, in_=S_new[:, 64:128],
                                             func=AF.Tanh, scale=0.5)
                        nc.vector.scalar_tensor_tensor(
                            out=H_new[:, 0:64], in0=TgA[:, 192:256], scalar=1.0,
                            in1=TcA, op0=ALU.add, op1=ALU.mult)
                        # ---- PE: logits bias + k=0,1 (sit after the
                        # prefill/init cover in the PE FIFO) ----
                        l_ps = l_psp.tile([BL, N], f32, tag="l")
                        if (ch == 0 and tt < 2) or not V3_BIAS_DVE:
                            nc.tensor.matmul(l_ps, ones32, bl, start=True,
                                             stop=False, skip_group_check=True)
                        else:
                            nc.vector.tensor_copy(out=l_ps, in_=blb)
                        for k in range(2):
                            nc.tensor.matmul(l_ps, H_new[:, 32 * k:32 * k + 32],
                                             wlt[:, N * k:N * (k + 1)],
                                             start=False, stop=False,
                                             skip_group_check=True)
                        nc.vector.scalar_tensor_tensor(
                            out=H_new[:, 64:128], in0=TgB[:, 192:256], scalar=1.0,
                            in1=TcB, op0=ALU.add, op1=ALU.mult)
                        # ---- PE: logits k=2,3 (need H_B) ----
                        for k in range(2, 4):
                            nc.tensor.matmul(l_ps, H_new[:, 32 * k:32 * k + 32],
                                             wlt[:, N * k:N * (k + 1)],
                                             start=False, stop=(k == 3),
                                             skip_group_check=True)
                        # ---- ACT/DVE: softmax -> y (fp16) -> yT feedback ----
                        e = tmp.tile([BL, N], f16, tag="e")
                        z = tmp.tile([BL, 1], f32, tag="z")
                        nc.scalar.activation(out=e, in_=l_ps, func=AF.Exp, scale=1.0,
                                             accum_out=z)
                        rz = tmp.tile([BL, 1], f32, tag="rz")
                        nc.vector.reciprocal(rz, z)
                        ysl = Y[:, tt * N:(tt + 1) * N]
                        nc.vector.tensor_scalar(out=ysl, in0=e, scalar1=rz,
                                                scalar2=None, op0=ALU.mult)
                        yT2 = state.tile([P, BL], f16, tag="yT")
                        nc.vector.transpose(out=yT2[0:32, :], in_=ysl[:, 0:32])
                        nc.vector.transpose(out=yT2[32:64, :], in_=ysl[:, 32:64])
                        if V3_WY_PAIRS:
                            nc.vector.tensor_copy(out=yT2[64:128, :],
                                                  in_=yT2[0:64, :])
                        if pp_fill is not None:
                            pre_copy(pp_fill, GXv, tt // 2, tt % 2)
                        S = S_new
                        H = H_new

                    # ---- flush Y chunk ----
                    nc.sync.dma_start(
                        out=out_d[:, ch * TC:(ch + 1) * TC, :].rearrange("b t n -> b (t n)"),
                        in_=Y,
                    )

    nc.compile()
    return nc


def _prep(W_ih, b_ih, W_hh, b_hh, W_lin, b_lin):
    sg = np.concatenate([
        np.full(D, 0.5), np.full(D, 0.5), np.ones(D), np.full(D, 0.5)
    ]).astype(np.float32)
    perm = np.array(_PERM)
    W_x = (W_ih[:, :D] * sg[:, None] * SCALE).astype(np.float32)
    W_y8 = (W_ih[:, D:] * sg[:, None] * SCALE).astype(ml_dtypes.float8_e4m3)
    W_h8 = (W_hh * sg[:, None] * 0.5 * SCALE).astype(ml_dtypes.float8_e4m3)
    b_g = ((b_ih + b_hh) * sg * SCALE).astype(np.float32)
    W_l2 = (W_lin * 0.5).astype(np.float16)

    wxt = (W_x.reshape(16, P, 4, P)[perm]
           .transpose(3, 2, 0, 1).reshape(P, 4 * 16 * P).copy())
    wht = (W_h8.reshape(16, P, 4, P)[perm]
           .transpose(3, 2, 0, 1).reshape(P, 4 * 16 * P).copy())
    # packed y-feedback weights: pair p stacks chunks (2p, 2p+1) as the
    # 0:64 / 64:128 row halves of one 128x128 stationary tile
    arrT = W_y8.reshape(16, P, N)[perm].transpose(0, 2, 1)  # [q, n, j]
    wyt = np.ascontiguousarray(arrT.transpose(1, 0, 2)).reshape(N, 16 * P)
    wyp = np.zeros((P, 8 * P), dtype=ml_dtypes.float8_e4m3)
    for p in range(8):
        wyp[0:64, p * P:(p + 1) * P] = arrT[2 * p]
        wyp[64:128, p * P:(p + 1) * P] = arrT[2 * p + 1]
    wlt = W_l2.reshape(N, 4, P).transpose(2, 1, 0).reshape(P, 4 * N).copy()
    bgt = b_g.reshape(16, P)[perm].T.copy()
    bl16 = b_lin.astype(np.float16).reshape(1, N)
    return dict(
        wxt=wxt, wht=wht, wyt=wyt, wyp=wyp, wlt=wlt, bgt=bgt,
        bl=bl16.copy(), blb=np.tile(bl16, (BL, 1)).copy(),
    )


def make_in_maps(x, init_h, W_ih, b_ih, W_hh, b_hh, W_lin, b_lin):
    x = np.asarray(x, dtype=np.float32)
    T = x.shape[1]
    assert x.shape == (B, T, D) and T % TC == 0
    shared = _prep(np.asarray(W_ih, np.float32), np.asarray(b_ih, np.float32),
                   np.asarray(W_hh, np.float32), np.asarray(b_hh, np.float32),
                   np.asarray(W_lin, np.float32), np.asarray(b_lin, np.float32))
    init_h = np.ascontiguousarray(np.asarray(init_h, np.float32))

    in_maps = []
    for i in range(NCORES):
        m = dict(shared)
        xc = x[i * BL:(i + 1) * BL]  # [BL, T, D]
        xc = xc.reshape(BL, T // TC, TC, 4, P).transpose(1, 4, 3, 0, 2)
        m["x"] = np.ascontiguousarray(xc).reshape(T // TC, P, 4 * BL * TC)
        m["h0"] = np.ascontiguousarray(init_h[i * BL:(i + 1) * BL])
        in_maps.append(m)
    return in_maps, T


def kernel(x, init_h, W_ih, b_ih, W_hh, b_hh, W_lin, b_lin, _trace=False):
    in_maps, T = make_in_maps(x, init_h, W_ih, b_ih, W_hh, b_hh, W_lin, b_lin)
    if T not in _CACHE:
        _CACHE[T] = _build(T)
    nc = _CACHE[T]

    res = run_bass_kernel_spmd(nc, in_maps, list(range(NCORES)), trace=_trace)
    out = np.concatenate(
        [res.results[i]["out"].astype(np.float32) for i in range(NCORES)], axis=0)
    if _trace:
        kernel.last_exec_time_ns = res.exec_time_ns
        kernel.last_results = res
    return out


# revision 40
# speedup vs baseline: 1.4291x; 1.4291x over previous
"""Trainium2 Bass kernel for an autoregressive LSTMCell decoder with softmax feedback.

Math (per timestep, PyTorch gate order i,f,g,o):
    gates = [x_t, y] @ W_ih.T + b_ih + h @ W_hh.T + b_hh
    i,f,o = sigmoid(...), g = tanh(...)
    c = f*c + i*g ; h = o*tanh(c) ; y = softmax(h @ W_lin.T + b_lin)

Strategy (8 NeuronCores, data parallel over batch, 32 sequences/core):
  * Features-on-partitions: gates.T [2048, 32] packed into one PSUM bank
    [128, (q,b)=512]; cell/hidden state packed [128, (k,b)=128].
  * Gate-chunk order q = (k//2)*8 + gate*2 + (k%2) so h-feature chunks
    k in {0,1} land in PSUM cols 0:256 ("A") and k in {2,3} in 256:512
    ("B").  The ACT/DVE tail (tanh -> cell update -> H) runs per half
    and pipelines against the PE: tanh_A runs while the PE still
    accumulates B columns, and the next step's W_hh k=0,1 matmuls start
    as soon as H_A exists.
  * Recurrent weights (W_hh, W_y) are fp8-e4m3 scaled by 64 (fast
    weight load = 4 elem/cycle; x64 keeps values out of the subnormal
    range).  The x64 is undone for free by the gate tanh's scale=1/64.
  * x @ W_x.T precomputed per 32-step chunk as full-utilization f32r
    matmuls; bias is folded in during the PSUM->SBUF staging copy
    (per-partition scalar add), result kept in SBUF as fp16 (x64).
  * All gate activations via Tanh (sigmoid(x)=0.5+0.5*tanh(x/2), scales
    folded into weights host-side); softmax via Exp + reciprocal -- both
    functions live in the single "exp_and_others" ACT table set.
  * Cell state carried as S=2c, hidden as H=2h (absorbed into
    0.5-scaled W_hh / W_lin).  State math in fp16 (2x DVE mode).
  * y written to DRAM in fp16; host converts to f32.
"""

import os
import sys

sys.path.insert(0, "/opt/trn_rl_repo")

# gx-init and logits bias moved off the PE via the PSUM has_written
# trick (DVE copy, emitted after H_B); row-tiled W_y crashes -- keep off.
V3_GX_ACT = os.environ.get("V3_GX_ACT", "1") == "1"
V3_WY_PAIRS = os.environ.get("V3_WY_PAIRS", "0") == "1"
V3_BIAS_DVE = os.environ.get("V3_BIAS_DVE", "1") == "1"
# Diagnostic: break the recurrence (matmuls read constant state) to
# measure pure PE-stream throughput without tail stalls.  WRONG RESULTS.
PE_ONLY = os.environ.get("PE_ONLY", "0") == "1"
# Issue the next step's PE identity-inject before logits k=2,3 so the PE
# has more covering work while the B-half tail (H_B) is still in flight.
INIT_EARLY = os.environ.get("INIT_EARLY", "1") == "1"

import numpy as np
import ml_dtypes

import concourse.bass as bass  # noqa: F401
import concourse.tile as tile
from concourse import bacc, mybir
from concourse.bass_utils import run_bass_kernel_spmd
from concourse.masks import make_identity

f32 = mybir.dt.float32
f32r = mybir.dt.float32r
f16 = mybir.dt.float16
f8 = mybir.dt.float8e4
AF = mybir.ActivationFunctionType
ALU = mybir.AluOpType

B, D, N = 256, 512, 64
NCORES = 8
BL = B // NCORES  # 32 sequences per core
TC = 32           # timesteps per chunk
P = 128
SCALE = 64.0      # fp8 weight scale, undone by tanh ACT scale

# chunk order: old chunk index m = gate*4 + k  ->  position q
# q = (k//2)*8 + gate*2 + (k%2):  A half (cols 0:256) holds k in {0,1}
_PERM = [0] * 16
for _q in range(16):
    _k = (_q // 8) * 2 + (_q % 2)
    _gate = (_q % 8) // 2
    _PERM[_q] = _gate * 4 + _k

_CACHE = {}


def _build(T, reps=1):
    NCH = T // TC
    nc = bacc.Bacc("TRN2", target_bir_lowering=False, debug=False, num_devices=NCORES)

    x_d = nc.dram_tensor("x", [NCH, P, 4 * BL * TC], f32, kind="ExternalInput").ap()
    h0_d = nc.dram_tensor("h0", [BL, D], f32, kind="ExternalInput").ap()
    wxt_d = nc.dram_tensor("wxt", [P, 4 * 16 * P], f32, kind="ExternalInput").ap()
    wht_d = nc.dram_tensor("wht", [P, 4 * 16 * P], f8, kind="ExternalInput").ap()
    wyt_d = nc.dram_tensor("wyt", [N, 16 * P], f8, kind="ExternalInput").ap()
    wyp_d = nc.dram_tensor("wyp", [P, 8 * P], f8, kind="ExternalInput").ap()
    wlt_d = nc.dram_tensor("wlt", [P, 4 * N], f16, kind="ExternalInput").ap()
    bgt_d = nc.dram_tensor("bgt", [P, 16], f32, kind="ExternalInput").ap()
    bl_d = nc.dram_tensor("bl", [1, N], f16, kind="ExternalInput").ap()
    blb_d = nc.dram_tensor("blb", [BL, N], f16, kind="ExternalInput").ap()
    out_d = nc.dram_tensor("out", [BL, T, N], f16, kind="ExternalOutput").ap()

    with tile.TileContext(nc) as tc:
        with (
            tc.tile_pool(name="const", bufs=1) as const,
            tc.tile_pool(name="xst", bufs=2) as xst,
            tc.tile_pool(name="gxp", bufs=2) as gxp,
            tc.tile_pool(name="yout", bufs=2) as yout,
            tc.tile_pool(name="state", bufs=2) as state,
            tc.tile_pool(name="tmp", bufs=3) as tmp,
            tc.tile_pool(name="g_ps", bufs=2, space="PSUM") as g_psp,
            tc.tile_pool(name="pre_ps", bufs=2, space="PSUM") as pre_psp,
            tc.tile_pool(name="l_ps", bufs=2, space="PSUM") as l_psp,
        ):
            # ---- constants / weights ----
            wxt = const.tile([P, 4 * 16 * P], f32r)
            nc.sync.dma_start(out=wxt, in_=wxt_d.bitcast(f32r))
            wht = const.tile([P, 4 * 16 * P], f8)
            nc.sync.dma_start(out=wht, in_=wht_d)
            wyt = const.tile([N, 16 * P], f8)
            nc.sync.dma_start(out=wyt, in_=wyt_d)
            wyp = const.tile([P, 8 * P], f8)
            nc.sync.dma_start(out=wyp, in_=wyp_d)
            wlt = const.tile([P, 4 * N], f16)
            nc.sync.dma_start(out=wlt, in_=wlt_d)
            bgt = const.tile([P, 16], f32)
            nc.sync.dma_start(out=bgt, in_=bgt_d)
            bl = const.tile([1, N], f16)
            nc.sync.dma_start(out=bl, in_=bl_d)
            blb = const.tile([BL, N], f16)
            nc.sync.dma_start(out=blb, in_=blb_d)

            ones32 = const.tile([1, BL], f16)
            nc.vector.memset(ones32, 1.0)
            idf = const.tile([P, P], f32)
            make_identity(nc, idf)
            id8 = const.tile([P, P], f8)
            nc.vector.tensor_copy(id8, idf)

            for _rep in range(reps):
                # ---- initial state ----
                h0s = const.tile([P, 4, BL], f32)
                h0r = h0_d.rearrange("b (k p) -> k p b", p=P)
                for k in range(4):
                    nc.sync.dma_start(out=h0s[:, k, :], in_=h0r[k])
                H = state.tile([P, 4 * BL], f16, tag="H")
                nc.vector.tensor_scalar(out=H, in0=h0s.rearrange("p k b -> p (k b)"),
                                        scalar1=2.0, scalar2=None, op0=ALU.mult)
                S = state.tile([P, 4 * BL], f16, tag="S")
                nc.vector.memset(S, 0.0)
                yT2 = state.tile([P, BL], f16, tag="yT")
                nc.vector.memset(yT2, 0.0)
                if PE_ONLY:
                    H0_const = const.tile([P, 4 * BL], f16)
                    nc.vector.memset(H0_const, 0.01)
                    yT2_const = const.tile([P, BL], f16)
                    nc.vector.memset(yT2_const, 0.01)

                def stage_x(ch):
                    xT = xst.tile([P, 4 * BL * TC], f32r, tag="xT")
                    nc.sync.dma_start(out=xT, in_=x_d[ch].bitcast(f32r))
                    return xT.rearrange("p (k bt) -> p k bt", k=4)

                def pre_mms(xTv, q, half):
                    # one PSUM tile of (64*sg*W_x) @ x for gate-chunk q,
                    # batch-half `half` (all 32 timesteps of the chunk)
                    pp = pre_psp.tile([P, 512], f32, tag="pre")
                    for kx in range(4):
                        nc.tensor.matmul(
                            pp,
                            wxt[:, (kx * 16 + q) * P:(kx * 16 + q + 1) * P],
                            xTv[:, kx, half * 512:(half + 1) * 512],
                            start=(kx == 0), stop=(kx == 3), skip_group_check=True,
                        )
                    return pp

                def pre_copy(pp, GXv, q, half):
                    # permuted copy into GX as fp16, bias folded in
                    ppv = pp.rearrange("c (b t) -> c t b", t=TC)
                    nc.vector.tensor_scalar(
                        out=GXv[:, :, 32 * q + 16 * half: 32 * q + 16 * half + 16],
                        in0=ppv, scalar1=bgt[:, q:q + 1], scalar2=None, op0=ALU.add)

                def pre_group(xTv, GXv, q, half):
                    pre_copy(pre_mms(xTv, q, half), GXv, q, half)

                xTv_next = stage_x(0)
                GX = gxp.tile([P, TC * 512], f16, tag="GX")
                GXv = GX.rearrange("p (t mm) -> p t mm", mm=512)
                for q in range(16):
                    for half in range(2):
                        pre_group(xTv_next, GXv, q, half)

                for ch in range(NCH):
                    GX_cur = GX
                    if ch + 1 < NCH:
                        xTv_next = stage_x(ch + 1)
                        GX = gxp.tile([P, TC * 512], f16, tag="GX")
                        GXv = GX.rearrange("p (t mm) -> p t mm", mm=512)
                    Y = yout.tile([BL, TC * N], f16, tag="Y")
                    if PE_ONLY:
                        nc.vector.memset(Y, 0.0)
                    g_ps_next = None

                    for tt in range(TC):
                        if PE_ONLY:
                            # constant stand-ins: PE stream only, no tails
                            H = H0_const
                            yT2 = yT2_const
                        # ---------- PE: accumulate gates for step tt ----------
                        # g_ps for this step was gx-initialized either by the
                        # PE identity-inject below (tt==0, or the very first
                        # bank occupancies) or by an ACT copy issued one step
                        # early; in the latter case the matmuls accumulate on
                        # top via the has_written bits still set from this
                        # bank's previous occupancy.
                        if g_ps_next is None:
                            g_ps = g_psp.tile([P, 512], f32, tag="g")
                            nc.tensor.matmul(g_ps, id8,
                                             GX_cur[:, tt * 512:(tt + 1) * 512],
                                             start=True, stop=False,
                                             skip_group_check=True)
                        else:
                            g_ps = g_ps_next
                            g_ps_next = None
                        # kpair-major: all k=0,1 matmuls (need only H_A of the
                        # previous step) for every output chunk, then k=2,3.
                        # Within each group A-columns (q<8) come first.  This
                        # pushes the last write of each column half ~1us
                        # earlier in the PE stream so the tanh/cell tails have
                        # more covering PE work.
                        for kpair in range(2):
                            for q in range(16):
                                for k in (2 * kpair, 2 * kpair + 1):
                                    nc.tensor.matmul(
                                        g_ps[:, 32 * q:32 * q + 32],
                                        wht[:, (k * 16 + q) * P:(k * 16 + q + 1) * P],
                                        H[:, 32 * k:32 * k + 32],
                                        start=False, stop=False,
                                        skip_group_check=True,
                                    )
                        # y feedback, A half then B half (latest dep: yT2)
                        for q in range(8):
                            nc.tensor.matmul(
                                g_ps[:, 32 * q:32 * q + 32],
                                wyt[:, q * P:(q + 1) * P],
                                yT2[0:64, :],
                                start=False, stop=(q == 7),
                                skip_group_check=True,
                            )
                        if not PE_ONLY:
                            # ---- ACT tail for the A half starts now ----
                            TgA = tmp.tile([P, 256], f16, tag="TgA")
                            nc.scalar.activation(out=TgA, in_=g_ps[:, 0:256],
                                                 func=AF.Tanh, scale=1.0 / SCALE)
                        for q in range(8, 16):
                            nc.tensor.matmul(
                                g_ps[:, 32 * q:32 * q + 32],
                                wyt[:, q * P:(q + 1) * P],
                                yT2[0:64, :],
                                start=False, stop=(q == 15),
                                skip_group_check=True,
                            )
                        # ---- logits bias (PSUM init; has_written trick) ----
                        l_ps = l_psp.tile([BL, N], f32, tag="l")
                        if (ch == 0 and tt < 2) or not V3_BIAS_DVE:
                            nc.tensor.matmul(l_ps, ones32, bl, start=True,
                                             stop=False, skip_group_check=True)
                        else:
                            nc.vector.tensor_copy(out=l_ps, in_=blb)
                        # ---- DVE: cell update, A half ----
                        if PE_ONLY:
                            for k in range(4):
                                nc.tensor.matmul(l_ps, H[:, 32 * k:32 * k + 32],
                                                 wlt[:, N * k:N * (k + 1)],
                                                 start=False, stop=(k == 3),
                                                 skip_group_check=True)
                            if ch + 1 < NCH:
                                pre_copy(pre_mms(xTv_next, tt // 2, tt % 2),
                                         GXv, tt // 2, tt % 2)
                            continue
                        S_new = state.tile([P, 4 * BL], f16, tag="S")
                        H_new = state.tile([P, 4 * BL], f16, tag="H")
                        uA = tmp.tile([P, 2 * BL], f16, tag="uA")
                        nc.vector.scalar_tensor_tensor(
                            out=uA, in0=TgA[:, 64:128], scalar=1.0,
                            in1=S[:, 0:64], op0=ALU.add, op1=ALU.mult)
                        vA = tmp.tile([P, 2 * BL], f16, tag="vA")
                        nc.vector.scalar_tensor_tensor(
                            out=vA, in0=TgA[:, 0:64], scalar=1.0,
                            in1=TgA[:, 128:192], op0=ALU.add, op1=ALU.mult)
                        nc.vector.scalar_tensor_tensor(
                            out=S_new[:, 0:64], in0=uA, scalar=0.5,
                            in1=vA, op0=ALU.mult, op1=ALU.add)
                        TcA = tmp.tile([P, 2 * BL], f16, tag="TcA")
                        nc.scalar.activation(out=TcA, in_=S_new[:, 0:64],
                                             func=AF.Tanh, scale=0.5)
                        nc.vector.scalar_tensor_tensor(
                            out=H_new[:, 0:64], in0=TgA[:, 192:256], scalar=1.0,
                            in1=TcA, op0=ALU.add, op1=ALU.mult)
                        # ---- PE: logits k=0,1 need only H_A ----
                        for k in range(2):
                            nc.tensor.matmul(l_ps, H_new[:, 32 * k:32 * k + 32],
                                             wlt[:, N * k:N * (k + 1)],
                                             start=False, stop=False,
                                             skip_group_check=True)
                        # ---- PE filler: next chunk's precompute MMs ----
                        pp_fill = None
                        if ch + 1 < NCH:
                            pp_fill = pre_mms(xTv_next, tt // 2, tt % 2)
                        # ---- gx-init the NEXT step's PSUM bank early ----
                        # (PE identity-inject path; the V3_GX_ACT DVE-copy
                        # variant is emitted after H_B so it cannot block the
                        # B chain in the DVE FIFO)
                        if (tt + 1 < TC and not (ch == 0 and tt == 0)
                                and not V3_GX_ACT and INIT_EARLY):
                            g_ps_next = g_psp.tile([P, 512], f32, tag="g")
                            nc.tensor.matmul(
                                g_ps_next, id8,
                                GX_cur[:, (tt + 1) * 512:(tt + 2) * 512],
                                start=True, stop=False,
                                skip_group_check=True)
                        # ---- ACT/DVE tail for the B half ----
                        TgB = tmp.tile([P, 256], f16, tag="TgB")
                        nc.scalar.activation(out=TgB, in_=g_ps[:, 256:512],
                                             func=AF.Tanh, scale=1.0 / SCALE)
                        uB = tmp.tile([P, 2 * BL], f16, tag="uB")
                        nc.vector.scalar_tensor_tensor(
                            out=uB, in0=TgB[:, 64:128], scalar=1.0,
                            in1=S[:, 64:128], op0=ALU.add, op1=ALU.mult)
                        vB = tmp.tile([P, 2 * BL], f16, tag="vB")
                        nc.vector.scalar_tensor_tensor(
                            out=vB, in0=TgB[:, 0:64], scalar=1.0,
                            in1=TgB[:, 128:192], op0=ALU.add, op1=ALU.mult)
                        nc.vector.scalar_tensor_tensor(
                            out=S_new[:, 64:128], in0=uB, scalar=0.5,
                            in1=vB, op0=ALU.mult, op1=ALU.add)
                        TcB = tmp.tile([P, 2 * BL], f16, tag="TcB")
                        nc.scalar.activation(out=TcB, in_=S_new[:, 64:128],
                                             func=AF.Tanh, scale=0.5)
                        nc.vector.scalar_tensor_tensor(
                            out=H_new[:, 64:128], in0=TgB[:, 192:256], scalar=1.0,
                            in1=TcB, op0=ALU.add, op1=ALU.mult)
                        if (V3_GX_ACT and tt + 1 < TC
                                and not (ch == 0 and tt == 0)):
                            g_ps_next = g_psp.tile([P, 512], f32, tag="g")
                            nc.vector.tensor_copy(
                                out=g_ps_next,
                                in_=GX_cur[:, (tt + 1) * 512:(tt + 2) * 512])
                        # ---- PE: logits k=2,3 (need H_B) ----
                        for k in range(2, 4):
                            nc.tensor.matmul(l_ps, H_new[:, 32 * k:32 * k + 32],
                                             wlt[:, N * k:N * (k + 1)],
                                             start=False, stop=(k == 3),
                                             skip_group_check=True)
                        # ---- ACT/DVE: softmax -> y (fp16) -> yT feedback ----
                        e = tmp.tile([BL, N], f16, tag="e")
                        z = tmp.tile([BL, 1], f32, tag="z")
                        nc.scalar.activation(out=e, in_=l_ps, func=AF.Exp, scale=1.0,
                                             accum_out=z)
                        rz = tmp.tile([BL, 1], f32, tag="rz")
                        nc.vector.reciprocal(rz, z)
                        ysl = Y[:, tt * N:(tt + 1) * N]
                        nc.vector.tensor_scalar(out=ysl, in0=e, scalar1=rz,
                                                scalar2=None, op0=ALU.mult)
                        yT2 = state.tile([P, BL], f16, tag="yT")
                        nc.vector.transpose(out=yT2[0:32, :], in_=ysl[:, 0:32])
                        nc.vector.transpose(out=yT2[32:64, :], in_=ysl[:, 32:64])
                        if V3_WY_PAIRS:
                            nc.vector.tensor_copy(out=yT2[64:128, :],
                                                  in_=yT2[0:64, :])
                        if pp_fill is not None:
                            pre_copy(pp_fill, GXv, tt // 2, tt % 2)
                        S = S_new
                        H = H_new

                    # ---- flush Y chunk ----
                    nc.sync.dma_start(
                        out=out_d[:, ch * TC:(ch + 1) * TC, :].rearrange("b t n -> b (t n)"),
                        in_=Y,
                    )

    nc.compile()
    return nc


def _prep(W_ih, b_ih, W_hh, b_hh, W_lin, b_lin):
    sg = np.concatenate([
        np.full(D, 0.5), np.full(D, 0.5), np.ones(D), np.full(D, 0.5)
    ]).astype(np.float32)
    perm = np.array(_PERM)
    W_x = (W_ih[:, :D] * sg[:, None] * SCALE).astype(np.float32)
    W_y8 = (W_ih[:, D:] * sg[:, None] * SCALE).astype(ml_dtypes.float8_e4m3)
    W_h8 = (W_hh * sg[:, None] * 0.5 * SCALE).astype(ml_dtypes.float8_e4m3)
    b_g = ((b_ih + b_hh) * sg * SCALE).astype(np.float32)
    W_l2 = (W_lin * 0.5).astype(np.float16)

    wxt = (W_x.reshape(16, P, 4, P)[perm]
           .transpose(3, 2, 0, 1).reshape(P, 4 * 16 * P).copy())
    wht = (W_h8.reshape(16, P, 4, P)[perm]
           .transpose(3, 2, 0, 1).reshape(P, 4 * 16 * P).copy())
    # packed y-feedback weights: pair p stacks chunks (2p, 2p+1) as the
    # 0:64 / 64:128 row halves of one 128x128 stationary tile
    arrT = W_y8.reshape(16, P, N)[perm].transpose(0, 2, 1)  # [q, n, j]
    wyt = np.ascontiguousarray(arrT.transpose(1, 0, 2)).reshape(N, 16 * P)
    wyp = np.zeros((P, 8 * P), dtype=ml_dtypes.float8_e4m3)
    for p in range(8):
        wyp[0:64, p * P:(p + 1) * P] = arrT[2 * p]
        wyp[64:128, p * P:(p + 1) * P] = arrT[2 * p + 1]
    wlt = W_l2.reshape(N, 4, P).transpose(2, 1, 0).reshape(P, 4 * N).copy()
    bgt = b_g.reshape(16, P)[perm].T.copy()
    bl16 = b_lin.astype(np.float16).reshape(1, N)
    return dict(
        wxt=wxt, wht=wht, wyt=wyt, wyp=wyp, wlt=wlt, bgt=bgt,
        bl=bl16.copy(), blb=np.tile(bl16, (BL, 1)).copy(),
    )


def make_in_maps(x, init_h, W_ih, b_ih, W_hh, b_hh, W_lin, b_lin):
    x = np.asarray(x, dtype=np.float32)
    T = x.shape[1]
    assert x.shape == (B, T, D) and T % TC == 0
    shared = _prep(np.asarray(W_ih, np.float32), np.asarray(b_ih, np.float32),
                   np.asarray(W_hh, np.float32), np.asarray(b_hh, np.float32),
                   np.asarray(W_lin, np.float32), np.asarray(b_lin, np.float32))
    init_h = np.ascontiguousarray(np.asarray(init_h, np.float32))

    in_maps = []
    for i in range(NCORES):
        m = dict(shared)
        xc = x[i * BL:(i + 1) * BL]  # [BL, T, D]
        xc = xc.reshape(BL, T // TC, TC, 4, P).transpose(1, 4, 3, 0, 2)
        m["x"] = np.ascontiguousarray(xc).reshape(T // TC, P, 4 * BL * TC)
        m["h0"] = np.ascontiguousarray(init_h[i * BL:(i + 1) * BL])
        in_maps.append(m)
    return in_maps, T


def kernel(x, init_h, W_ih, b_ih, W_hh, b_hh, W_lin, b_lin, _trace=False):
    in_maps, T = make_in_maps(x, init_h, W_ih, b_ih, W_hh, b_hh, W_lin, b_lin)
    if T not in _CACHE:
        _CACHE[T] = _build(T)
    nc = _CACHE[T]

    res = run_bass_kernel_spmd(nc, in_maps, list(range(NCORES)), trace=_trace)
    out = np.concatenate(
        [res.results[i]["out"].astype(np.float32) for i in range(NCORES)], axis=0)
    if _trace:
        kernel.last_exec_time_ns = res.exec_time_ns
        kernel.last_results = res
    return out
